# revision 1
# baseline (speedup 1.0000x reference)
"""DeepSeek MoE gate (sigmoid routing, grouped top-k) for 8x Trainium2 NeuronCores.

Strategy: data-parallel over tokens (16384 tokens -> 2048 per core), gate
weight + bias replicated. Per core:
  - stream x tiles [128, 7168] fp32 from HBM,
  - PE-transpose them (fp32) into PSUM,
  - split into bf16 hi/lo directly out of PSUM (hi = cast on ACT,
    lo = residual subtract on DVE) -> xh, xl bf16 in [h, t] layout,
  - per 128-h-chunk: 2 bf16 matmuls accumulate logits in one PSUM bank:
      xh @ [Wh | Wl]  (N=512)  and  xl @ Wh  (N=256, into low half)
    so logits = xh@Wh + xh@Wl + xl@Wh (+O(2^-18) dropped xl@Wl term),
    reconstructed as A[:, :256] + A[:, 256:] on DVE,
  - sigmoid on ACT; +bias, grouped max, native top-8 (InstMax/InstMaxIndex),
    normalize on DVE; weights + int indices out.
"""

import os
import sys

sys.path.insert(0, "/opt/trn_rl_repo")

import numpy as np

import concourse.bass as bass
import concourse.mybir as mybir
import concourse.tile as tile
from concourse.bass_utils import run_bass_kernel_spmd
from concourse.masks import make_identity

P = 128
H = 7168
E = 256
G = 8  # n_group
GSZ = E // G  # 32 experts per group
TOPK_G = 4
TOPK = 8
N_CORES = 8
T_FULL = 4 * 4096
T_CORE = T_FULL // N_CORES
HC = H // P  # 56 contraction chunks

F32 = mybir.dt.float32
BF16 = mybir.dt.bfloat16
U32 = mybir.dt.uint32

TCH = 7  # transpose chunks per PSUM staging tile (fits 2 banks)
H2 = H // 2  # h-half tile size (3584)
HC2 = HC // 2  # chunks per half (28)


def build_moe_gate(tc: tile.TileContext, x_d, w_d, b_d, wout_d, iout_d, t_core,
                   ctx=None):
    nc = tc.nc
    nt = t_core // P

    const_pool = ctx.enter_context(tc.tile_pool(name="const", bufs=1))
    xin_pool = ctx.enter_context(tc.tile_pool(name="xin", bufs=4))
    xhl_pool = ctx.enter_context(tc.tile_pool(name="xhl", bufs=2))
    ps_t_pool = ctx.enter_context(tc.tile_pool(name="ps_t", bufs=3, space="PSUM"))
    ps_l_pool = ctx.enter_context(tc.tile_pool(name="ps_l", bufs=2, space="PSUM"))
    sc_pool = ctx.enter_context(tc.tile_pool(name="scores", bufs=2))
    sm_pool = ctx.enter_context(tc.tile_pool(name="small", bufs=4))
    out_pool = ctx.enter_context(tc.tile_pool(name="outs", bufs=1))

    identity = const_pool.tile([P, P], F32)
    make_identity(nc, identity)

    # bias replicated across partitions: [128, 256]
    bias_rep = const_pool.tile([P, E], F32)
    nc.sync.dma_start(bias_rep, b_d[None, :].to_broadcast([P, E]))

    # W split: Whl[:, j, 0:256] = Wh chunk j, Whl[:, j, 256:512] = Wl chunk j
    whl = const_pool.tile([P, HC, 2 * E], BF16)

    def transpose_split(src_sb, j0, njc, dst_hi_fn, dst_lo_fn):
        """PE-transpose njc fp32 [128,128] chunks of src_sb (chunk j0+local
        globally), then split each PSUM staging unit into bf16 hi (ACT cast)
        + lo (DVE residual). dst fns take (global_j0, n)."""
        for b0 in range(0, njc, TCH):
            n = min(TCH, njc - b0)
            pt = ps_t_pool.tile([P, TCH, P], F32, tag="ps_t")
            for q in range(n):
                jl = b0 + q
                nc.tensor.matmul(pt[:, q, :], src_sb[:, jl * P:(jl + 1) * P],
                                 identity, is_transpose=True,
                                 start=(q % 4 == 0), stop=(q % 4 == 3 or q == n - 1))
            hi = dst_hi_fn(j0 + b0, n)
            nc.scalar.copy(hi, pt[:, :n, :])
            nc.vector.tensor_sub(dst_lo_fn(j0 + b0, n), pt[:, :n, :], hi)

    def load_x_half(i, hf):
        x_sb = xin_pool.tile([P, H2], F32, tag="xin")
        nc.sync.dma_start(x_sb, x_d[i * P:(i + 1) * P, hf * H2:(hf + 1) * H2])
        return x_sb

    def split_x_tile(halves, xh, xl):
        for hf in (0, 1):
            transpose_split(halves[hf], hf * HC2, HC2,
                            lambda j0, n: xh[:, j0:j0 + n, :],
                            lambda j0, n: xl[:, j0:j0 + n, :])

    # prefetch + transpose tile 0 ahead of the W DMAs so PE starts immediately
    x0_halves = (load_x_half(0, 0), load_x_half(0, 1))
    xh0 = xhl_pool.tile([P, HC, P], BF16, tag="xh")
    xl0 = xhl_pool.tile([P, HC, P], BF16, tag="xl")
    split_x_tile(x0_halves, xh0, xl0)
    xhl_prefetch = {0: (xh0, xl0)}

    # ---- build Wh/Wl (one-time; W DMAs ride the ACT HWDGE ring) ----
    for e2 in range(E // P):
        for hf in (0, 1):
            w_sl = xin_pool.tile([P, H2], F32, tag="xin")
            nc.scalar.dma_start(
                w_sl, w_d[e2 * P:(e2 + 1) * P, hf * H2:(hf + 1) * H2])
            transpose_split(
                w_sl, hf * HC2, HC2,
                lambda j0, n, e2=e2: whl[:, j0:j0 + n, e2 * P:(e2 + 1) * P],
                lambda j0, n, e2=e2: whl[:, j0:j0 + n,
                                         E + e2 * P:E + (e2 + 1) * P],
            )

    wout_sb = out_pool.tile([P, nt, TOPK], F32)
    iout_sb = out_pool.tile([P, nt, TOPK], U32)

    def emit_tr_unit(halves, xh, xl, u):
        """One transpose staging unit (7 chunks) for the next tile: PE
        transposes + ACT hi-cast + DVE lo-residual."""
        hf = u // 4
        b0 = (u % 4) * TCH
        j0 = hf * HC2 + b0
        pt = ps_t_pool.tile([P, TCH, P], F32, tag="ps_t")
        for q in range(TCH):
            jl = b0 + q
            nc.tensor.matmul(pt[:, q, :],
                             halves[hf][:, jl * P:(jl + 1) * P],
                             identity, is_transpose=True,
                             start=(q % 4 == 0), stop=(q % 4 == 3 or q == TCH - 1))
        hi = xh[:, j0:j0 + TCH, :]
        nc.scalar.copy(hi, pt)
        nc.vector.tensor_sub(xl[:, j0:j0 + TCH, :], pt, hi)

    # ---- main loop over token tiles (software-pipelined: tile i+1's
    # transpose units interleave with tile i's matmul chunks so DVE's
    # residual-subtracts stay ahead of PE's PSUM-slot reuse) ----
    for i in range(nt):
        xh, xl = xhl_prefetch.pop(i)
        if i + 1 < nt:
            nxt_halves = (load_x_half(i + 1, 0), load_x_half(i + 1, 1))
            nxh = xhl_pool.tile([P, HC, P], BF16, tag="xh")
            nxl = xhl_pool.tile([P, HC, P], BF16, tag="xl")
            xhl_prefetch[i + 1] = (nxh, nxl)

        lg = ps_l_pool.tile([P, 2 * E], F32, tag="ps_l")
        for u in range(HC // TCH):
            if i + 1 < nt:
                emit_tr_unit(nxt_halves, nxh, nxl, u)
            for j in range(u * TCH, (u + 1) * TCH):
                nc.tensor.matmul(lg, xh[:, j, :], whl[:, j, :],
                                 start=(j == 0), stop=False)
                nc.tensor.matmul(lg[:, :E], xl[:, j, :], whl[:, j, :E],
                                 start=False, stop=(j == HC - 1))

        # logits = A[:, :256] + A[:, 256:512]; scores = sigmoid(logits) + bias
        # (DVE cannot read two PSUM operands: stage the hi half via ACT)
        lg_hi = sc_pool.tile([P, E], F32, tag="lg_hi")
        nc.scalar.copy(lg_hi, lg[:, E:])
        logits = sc_pool.tile([P, E], F32, tag="logits")
        nc.vector.tensor_add(logits, lg[:, :E], lg_hi)
        scores = sc_pool.tile([P, E], F32, tag="scores")
        nc.scalar.activation(scores, logits, mybir.ActivationFunctionType.Sigmoid)
        nc.gpsimd.tensor_add(scores, scores, bias_rep)

        scores_g = scores.rearrange("p (g e) -> p g e", g=G)
        gmax = sm_pool.tile([P, G], F32, tag="gmax")
        nc.vector.reduce_max(gmax, scores_g, axis=mybir.AxisListType.X)

        g8 = sm_pool.tile([P, 8], F32, tag="g8")
        nc.vector.max(out=g8, in_=gmax)

        # group mask: 1.0 where group score >= 4th-largest group score
        gmask = sm_pool.tile([P, G], F32, tag="gmask")
        nc.vector.tensor_scalar(gmask, gmax, g8[:, TOPK_G - 1:TOPK_G], None,
                                op0=mybir.AluOpType.is_ge)

        masked = sc_pool.tile([P, E], F32, tag="masked")
        nc.gpsimd.tensor_tensor(
            masked.rearrange("p (g e) -> p g e", g=G), scores_g,
            gmask[:, :, None].to_broadcast([P, G, GSZ]),
            op=mybir.AluOpType.mult)

        m8 = sm_pool.tile([P, 8], F32, tag="m8")
        nc.vector.max(out=m8, in_=masked)
        nc.vector.max_index(iout_sb[:, i, :], m8, masked)

        ssum = sm_pool.tile([P, 1], F32, tag="ssum")
        nc.vector.reduce_sum(ssum, m8, axis=mybir.AxisListType.X)
        nc.vector.tensor_scalar_add(ssum, ssum, 1e-6)
        rcp = sm_pool.tile([P, 1], F32, tag="rcp")
        nc.vector.reciprocal(rcp, ssum)
        nc.vector.tensor_scalar_mul(wout_sb[:, i, :], m8, rcp)

    # outputs in [p, n, k] layout; host reorders to [n*p, k]
    nc.sync.dma_start(wout_d, wout_sb)
    nc.sync.dma_start(iout_d, iout_sb)


def build_bass(t_core=T_CORE):
    from concourse import bacc
    nc = bacc.Bacc("TRN2", target_bir_lowering=False, debug=False,
                   num_devices=N_CORES)
    nt = t_core // P
    x_d = nc.dram_tensor("x", [t_core, H], F32, kind="ExternalInput").ap()
    w_d = nc.dram_tensor("w", [E, H], F32, kind="ExternalInput").ap()
    b_d = nc.dram_tensor("b", [E], F32, kind="ExternalInput").ap()
    wout_d = nc.dram_tensor("wout", [P, nt, TOPK], F32,
                            kind="ExternalOutput").ap()
    iout_d = nc.dram_tensor("iout", [P, nt, TOPK], U32,
                            kind="ExternalOutput").ap()
    from contextlib import ExitStack
    with tile.TileContext(nc) as tc:
        with ExitStack() as ctx:
            build_moe_gate(tc, x_d, w_d, b_d, wout_d, iout_d, t_core, ctx=ctx)
    nc.compile()
    return nc


_NC_CACHE = {}


def _get_nc():
    key = "main"
    if key not in _NC_CACHE:
        _NC_CACHE[key] = build_bass()
    return _NC_CACHE[key]


def kernel(hidden_states, gate_weight, bias, n_group, topk_group, top_k,
           _trace=False):
    assert int(n_group) == G and int(topk_group) == TOPK_G and int(top_k) == TOPK
    x = np.asarray(hidden_states, dtype=np.float32)
    w = np.ascontiguousarray(np.asarray(gate_weight, dtype=np.float32))
    b = np.ascontiguousarray(np.asarray(bias, dtype=np.float32))
    B, S, _ = x.shape
    xf = x.reshape(-1, H)
    assert xf.shape[0] == T_FULL

    nc = _get_nc()
    in_maps = []
    for c in range(N_CORES):
        in_maps.append({
            "x": np.ascontiguousarray(xf[c * T_CORE:(c + 1) * T_CORE]),
            "w": w,
            "b": b,
        })
    try:
        res = run_bass_kernel_spmd(nc, in_maps, core_ids=list(range(N_CORES)),
                                   trace=_trace)
    except ModuleNotFoundError:
        # axon NTFF profiling hook unavailable in this container
        res = run_bass_kernel_spmd(nc, in_maps, core_ids=list(range(N_CORES)),
                                   trace=False)
    weights = np.empty((T_FULL, TOPK), dtype=np.float32)
    indices = np.empty((T_FULL, TOPK), dtype=np.int32)
    for c, r in enumerate(res.results):
        # [P, nt, K] -> [nt*P, K]
        wc = np.transpose(r["wout"], (1, 0, 2)).reshape(T_CORE, TOPK)
        ic = np.transpose(r["iout"], (1, 0, 2)).reshape(T_CORE, TOPK)
        weights[c * T_CORE:(c + 1) * T_CORE] = wc
        indices[c * T_CORE:(c + 1) * T_CORE] = ic.astype(np.int32)
    out_w = weights.reshape(B, S, TOPK)
    out_i = indices.reshape(B, S, TOPK)
    if _trace:
        return (out_w, out_i), res
    return out_w, out_i

